# revision 1
# baseline (speedup 1.0000x reference)
"""Multi-head attention (B=4, S=2048, E=1024, H=16) on 8 TRN2 NeuronCores.

Sharding: batch x query-half data parallel -- core c handles batch c//2,
query rows [ (c%2)*1024 : (c%2+1)*1024 ].  Each core computes K/V for its
full batch (redundant KV projection, no collectives needed), runs all 16
heads of attention for its 1024 query rows, and the output projection.

Layout choices (all chosen so no on-chip transposes are needed):
  - x arrives pre-transposed from host as x^T [E, S] with the core's own
    query half first in the S order (attention is permutation-invariant
    along keys, so K/V use the same reordered S).
  - Q^T, K^T [d, s] produced by matmul(lhsT=W_slice, rhs=x^T).
  - scores computed transposed: S^T[k, q] = matmul(lhsT=K^T_tile, rhs=Q^T).
  - softmax denominator via a ones-column appended to V: the PV matmul
    (lhsT=V_aug [k,65], rhs=exp(S^T)) yields rows 0-63 = (P V)^T and
    row 64 = per-query sums, accumulated over k-tiles in PSUM for free.
  - output projection computed transposed: out^T = matmul(lhsT=W_out,
    rhs=SC^T); host transposes the [E, q] result when reassembling.

Compute dtype bf16 (weights/activations), fp32 PSUM accumulation, exp on
ScalarE in fp32 from PSUM.
"""

import sys

if "/opt/trn_rl_repo" not in sys.path:
    sys.path.insert(0, "/opt/trn_rl_repo")

import numpy as np
import ml_dtypes

B, S, E, H = 4, 2048, 1024, 16
HD = E // H  # 64
N_CORES = 8
QH = S // 2  # query rows per core (1024)
P = 128
ET = E // P  # 8 e-tiles
ST = S // P  # 16 s-tiles
QC = QH // 512  # 2 q chunks of 512

_BF16 = ml_dtypes.bfloat16

_cached = None  # (nc, run_fn)

DEBUG_DUMPS = False
REPEAT = 1
SKIP_NORM = False
NO_PBCAST = False
NO_EXTRACT = False
NO_RECIP = False
DUMP_P = False


def _build():
    import concourse.bass as bass
    import concourse.tile as tile
    import concourse.mybir as mybir
    from concourse import bacc

    dt = mybir.dt
    nc = bacc.Bacc("TRN2", target_bir_lowering=False, debug=False)

    xt_d = nc.dram_tensor("xt", [E, S], dt.bfloat16, kind="ExternalInput").ap()
    wq_d = nc.dram_tensor("wq", [E, E], dt.bfloat16, kind="ExternalInput").ap()
    wk_d = nc.dram_tensor("wk", [E, E], dt.bfloat16, kind="ExternalInput").ap()
    wv_d = nc.dram_tensor("wv", [E, E], dt.bfloat16, kind="ExternalInput").ap()
    wo_d = nc.dram_tensor("wo", [E, E], dt.bfloat16, kind="ExternalInput").ap()
    bq_d = nc.dram_tensor("bq", [P, ET], dt.float32, kind="ExternalInput").ap()
    bk_d = nc.dram_tensor("bk", [P, ET], dt.float32, kind="ExternalInput").ap()
    bv_d = nc.dram_tensor("bv", [1, E], dt.bfloat16, kind="ExternalInput").ap()
    bo_d = nc.dram_tensor("bo", [P, ET], dt.float32, kind="ExternalInput").ap()
    out_d = nc.dram_tensor("out", [E, QH], dt.float32, kind="ExternalOutput").ap()
    if DUMP_P:
        dp_d = nc.dram_tensor("dp", [H, ST, P, QH], dt.bfloat16, kind="ExternalOutput").ap()
    if DEBUG_DUMPS:
        dqt_d = nc.dram_tensor("dqt", [P, ET * QH], dt.bfloat16, kind="ExternalOutput").ap()
        dkt_d = nc.dram_tensor("dkt", [P, ET * S], dt.bfloat16, kind="ExternalOutput").ap()
        dva_d = nc.dram_tensor("dva", [P, ST * H * (HD + 1)], dt.bfloat16, kind="ExternalOutput").ap()
        dscb_d = nc.dram_tensor("dscb", [P, ET * QH], dt.bfloat16, kind="ExternalOutput").ap()

    SCALE = 1.0 / float(np.sqrt(HD))

    with tile.TileContext(nc) as tc:
        with (
            tc.tile_pool(name="const", bufs=1) as cpool,
            tc.tile_pool(name="acts", bufs=1) as apool,
            tc.tile_pool(name="work", bufs=2) as wpool,
            tc.tile_pool(name="norm", bufs=1) as npool,
            tc.tile_pool(name="norm2", bufs=2) as n2pool,
            tc.tile_pool(name="bcp", bufs=1) as bcpool,
            tc.tile_pool(name="wv2", bufs=2) as wv2pool,
        ):
          with (tc.For_i(0, REPEAT, 1) if REPEAT > 1 else __import__("contextlib").nullcontext()):
              xt = cpool.tile([P, ET, S], dt.bfloat16)
              wo = cpool.tile([P, ET, E], dt.bfloat16)
              bq = cpool.tile([P, ET], dt.float32)
              bk = cpool.tile([P, ET], dt.float32)
              bv = cpool.tile([1, E], dt.bfloat16)
              bo = cpool.tile([P, ET], dt.float32)
              ones1 = cpool.tile([1, P], dt.bfloat16)

              nc.sync.dma_start(bq[:], bq_d)
              nc.sync.dma_start(bk[:], bk_d)
              nc.sync.dma_start(bv[:], bv_d)
              nc.sync.dma_start(bo[:], bo_d)
              wvcs = []
              for c in range(2):
                  wvc = wv2pool.tile(
                      [P, ET, 512], dt.bfloat16, tag="wvc", name=f"wvc{c}"
                  )
                  nc.sync.dma_start(
                      wvc[:],
                      wv_d[:, c * 512 : (c + 1) * 512].rearrange(
                          "(eo p) c -> p eo c", p=P
                      ),
                  )
                  wvcs.append(wvc)
              for sx in range(4):
                  nc.sync.dma_start(
                      xt[:, :, sx * 512 : (sx + 1) * 512],
                      xt_d[:, sx * 512 : (sx + 1) * 512].rearrange(
                          "(eo p) s -> p eo s", p=P
                      ),
                  )
              nc.sync.dma_start(wo[:], wo_d.rearrange("(eo p) c -> p eo c", p=P))
              nc.gpsimd.memset(ones1[:], 1.0)

              qt = apool.tile([P, ET, QH], dt.bfloat16)   # Q^T + bq
              kt = apool.tile([P, ET, S], dt.bfloat16)    # K^T + bk
              va = apool.tile([P, ST, H, HD + 1], dt.bfloat16)  # V (+bias) | ones col
              scb = apool.tile([P, ET, QH], dt.bfloat16)  # normalized SC^T

              nc.vector.memset(va[:, :, :, HD : HD + 1], 1.0)

              # ---- interleaved phases 1+2: V projection first, then per
              # head-pair: its Q/K projection immediately followed by its
              # attention, all sharing one PSUM pool so the PE never drains.
              ph1 = tc.tile_pool(name="psA", bufs=2, space="PSUM")
              ph2 = tc.tile_pool(name="psB", bufs=4, space="PSUM")
              pspool = ph1.__enter__()   # tag "sc": [128,1024] 2-bank tiles
              ps4pool = ph2.__enter__()  # tag "sm": [128,512] 1-bank tiles

              # V projection: V[s, d] for all heads (+bias via K=1 matmul)
              for c in range(2):
                  wvc = wvcs[c]
                  for st in range(ST):
                      ps = ps4pool.tile([P, 512], dt.float32, tag="sm", name=f"psv{st}{c}")
                      for e in range(ET):
                          nc.tensor.matmul(
                              ps[:],
                              xt[:, e, st * P : (st + 1) * P],
                              wvc[:, e, :],
                              start=(e == 0),
                              stop=False,
                          )
                      nc.tensor.matmul(
                          ps[:],
                          ones1[0:1, :],
                          bv[0:1, c * 512 : (c + 1) * 512],
                          start=False,
                          stop=True,
                      )
                      nc.vector.tensor_copy(
                          va[:, st, c * 8 : (c + 1) * 8, 0:HD],
                          ps.rearrange("p (h d) -> p h d", d=HD),
                      )

              for t in range(ET):
                  # Q^T, K^T projection for this pair's head tile t
                  wqt = wpool.tile([P, ET, P], dt.bfloat16, tag="wt", name=f"wqt{t}")
                  nc.sync.dma_start(
                      wqt[:],
                      wq_d[:, t * P : (t + 1) * P].rearrange("(eo p) c -> p eo c", p=P),
                  )
                  psq = pspool.tile([P, 1024], dt.float32, tag="sc", name=f"psq{t}")
                  for c in range(QC):
                      for e in range(ET):
                          nc.tensor.matmul(
                              psq[:, c * 512 : (c + 1) * 512],
                              wqt[:, e, :],
                              xt[:, e, c * 512 : (c + 1) * 512],
                              start=(e == 0),
                              stop=(e == ET - 1),
                          )
                  nc.vector.tensor_scalar_add(qt[:, t, :], psq[:], bq[:, t : t + 1])
                  wkt = wpool.tile([P, ET, P], dt.bfloat16, tag="wt", name=f"wkt{t}")
                  nc.sync.dma_start(
                      wkt[:],
                      wk_d[:, t * P : (t + 1) * P].rearrange("(eo p) c -> p eo c", p=P),
                  )
                  for ck in range(2):
                      psk = pspool.tile([P, 1024], dt.float32, tag="sc", name=f"psk{t}{ck}")
                      for c in range(2):
                          for e in range(ET):
                              nc.tensor.matmul(
                                  psk[:, c * 512 : (c + 1) * 512],
                                  wkt[:, e, :],
                                  xt[:, e, (ck * 2 + c) * 512 : (ck * 2 + c + 1) * 512],
                                  start=(e == 0),
                                  stop=(e == ET - 1),
                              )
                      nc.vector.tensor_scalar_add(
                          kt[:, t, ck * 1024 : (ck + 1) * 1024], psk[:], bk[:, t : t + 1]
                      )

                  # attention for heads (2t, 2t+1), interleaved in the PE array
                  pv = [
                      ps4pool.tile([P, 512], dt.float32, tag="sm", name=f"pv{t}{i}")
                      for i in range(4)  # [even c0, even c1, odd c0, odd c1]
                  ]
                  for j in range(ST):
                      for c in range(QC):
                          sc = pspool.tile(
                              [P, 1024], dt.float32, tag="sc", name=f"sc{t}{j}{c}"
                          )
                          nc.tensor.matmul(
                              sc[:, 0:512],
                              kt[0:64, t, j * P : (j + 1) * P],
                              qt[0:64, t, c * 512 : (c + 1) * 512],
                              start=True,
                              stop=True,
                          )
                          nc.tensor.matmul(
                              sc[:, 512:1024],
                              kt[64:128, t, j * P : (j + 1) * P],
                              qt[64:128, t, c * 512 : (c + 1) * 512],
                              start=True,
                              stop=True,
                          )
                          p = wpool.tile([P, 1024], dt.bfloat16, tag="p")
                          nc.scalar.activation(
                              p[:], sc[:], mybir.ActivationFunctionType.Exp, scale=SCALE
                          )
                          if DUMP_P:
                              nc.sync.dma_start(dp_d[2 * t, j, :, c * 512 : (c + 1) * 512], p[:, 0:512])
                              nc.sync.dma_start(dp_d[2 * t + 1, j, :, c * 512 : (c + 1) * 512], p[:, 512:1024])
                          nc.tensor.matmul(
                              pv[c][0 : HD + 1, :],
                              va[:, j, 2 * t, :],
                              p[:, 0:512],
                              start=(j == 0),
                              stop=(j == ST - 1),
                          )
                          nc.tensor.matmul(
                              pv[2 + c][0 : HD + 1, :],
                              va[:, j, 2 * t + 1, :],
                              p[:, 512:1024],
                              start=(j == 0),
                              stop=(j == ST - 1),
                          )
                  # fast raw evacuation releases the PSUM slots; normalize after
                  for half in range(2):
                      h = 2 * t + half
                      hp = half * 64
                      pvr = n2pool.tile([64, QH], dt.bfloat16, tag="pvr", name=f"pvr{h}")
                      srow_t = n2pool.tile([1, QH], dt.float32, tag="srow", name=f"sr{h}")
                      scr_t = npool.tile([1, QH], dt.float32, tag="scr", name=f"sx{h}")
                      rrow_t = npool.tile([1, QH], dt.float32, tag="rrow", name=f"rr{h}")
                      srow, scr, rrow = srow_t[0:1, :], scr_t[0:1, :], rrow_t[0:1, :]
                      for c in range(QC):
                          nc.vector.tensor_copy(
                              pvr[:, c * 512 : (c + 1) * 512],
                              pv[2 * half + c][0:HD, :],
                          )
                          nc.vector.tensor_copy(
                              srow[0:1, c * 512 : (c + 1) * 512],
                              pv[2 * half + c][HD : HD + 1, :],
                          )
                      nc.vector.reciprocal_approx_accurate(rrow, srow, scr)
                      bc = bcpool.tile([64, QH], dt.float32, tag="bc", name=f"bc{h}")
                      nc.gpsimd.partition_broadcast(bc[:], rrow[0:1, :])
                      nc.vector.tensor_tensor(
                          scb[hp : hp + HD, t, :],
                          pvr[:],
                          bc[:],
                          mybir.AluOpType.mult,
                      )
              # ---- phase 3: output projection (transposed) + bias
              # reuses the "sc" psum tag so its slots rotate in as exp frees them
              for t2 in range(ET):
                  ps = pspool.tile([P, 1024], dt.float32, tag="sc", name=f"pso{t2}")
                  for c in range(QC):
                      for e in range(ET):
                          nc.tensor.matmul(
                              ps[:, c * 512 : (c + 1) * 512],
                              wo[:, e, t2 * P : (t2 + 1) * P],
                              scb[:, e, c * 512 : (c + 1) * 512],
                              start=(e == 0),
                              stop=(e == ET - 1),
                          )
                  ot = wpool.tile([P, QH], dt.float32, tag="ot", name=f"ot{t2}")
                  nc.vector.tensor_scalar_add(ot[:], ps[:], bo[:, t2 : t2 + 1])
                  nc.sync.dma_start(out_d[t2 * P : (t2 + 1) * P, :], ot[:])
              ph2.__exit__(None, None, None)
              ph1.__exit__(None, None, None)

    nc.compile()
    return nc


def _prep_inputs(x, W_qkv, b_qkv, W_out, b_out):
    """Host-side sharding + layout prep. Returns per-core input maps."""
    w = W_qkv.reshape(E, H, 3, HD)
    wq = np.ascontiguousarray(w[:, :, 0, :].reshape(E, E)).astype(_BF16)
    wk = np.ascontiguousarray(w[:, :, 1, :].reshape(E, E)).astype(_BF16)
    wv = np.ascontiguousarray(w[:, :, 2, :].reshape(E, E)).astype(_BF16)
    wo = W_out.astype(_BF16)
    b3 = b_qkv.reshape(H, 3, HD)
    bq = np.ascontiguousarray(b3[:, 0, :].reshape(ET, P).T).astype(np.float32)
    bk = np.ascontiguousarray(b3[:, 1, :].reshape(ET, P).T).astype(np.float32)
    bv = np.ascontiguousarray(b3[:, 2, :].reshape(1, E)).astype(_BF16)
    bo = np.ascontiguousarray(b_out.reshape(ET, P).T).astype(np.float32)

    in_maps = []
    for core in range(N_CORES):
        b, half = core // 2, core % 2
        xb = x[b]  # [S, E]
        order = np.r_[half * QH : (half + 1) * QH, (1 - half) * QH : (2 - half) * QH]
        xt = np.ascontiguousarray(xb[order].T).astype(_BF16)  # [E, S], own q first
        in_maps.append(
            {
                "xt": xt,
                "wq": wq,
                "wk": wk,
                "wv": wv,
                "wo": wo,
                "bq": bq,
                "bk": bk,
                "bv": bv,
                "bo": bo,
            }
        )
    return in_maps


def run_raw(x, W_qkv, b_qkv, W_out, b_out, trace=False, **kw):
    """Run on hardware; returns (full_output [B,S,E] f32, BassKernelResults)."""
    global _cached
    from concourse.bass_utils import run_bass_kernel_spmd

    if _cached is None:
        _cached = _build()
    nc = _cached
    in_maps = _prep_inputs(x, W_qkv, b_qkv, W_out, b_out)
    res = run_bass_kernel_spmd(
        nc, in_maps, core_ids=list(range(N_CORES)), trace=trace, **kw
    )
    out = np.empty((B, S, E), dtype=np.float32)
    for core in range(N_CORES):
        b, half = core // 2, core % 2
        out[b, half * QH : (half + 1) * QH, :] = np.asarray(
            res.results[core]["out"]
        ).T
    return out, res


def kernel(x, W_qkv, b_qkv, W_out, b_out):
    out, _ = run_raw(x, W_qkv, b_qkv, W_out, b_out, trace=False)
    return out



# revision 12
# speedup vs baseline: 1.2130x; 1.2130x over previous
"""Multi-head attention (B=4, S=2048, E=1024, H=16) on 8 TRN2 NeuronCores.

Sharding: batch x head-group tensor parallel -- core c = 2*b + hg handles
batch b and heads hg*8 .. hg*8+7 for ALL 2048 queries.  Q/K/V projections
are column-split by head (each core projects only its 8 heads); the output
projection is row-split (each core contracts its 512 E-rows of W_out) and
produces a partial [E, S] output that the HOST sums across the core pair
while unsharding (the "all-reduce" of the sharding hint, done on host).

Kernel structure per core:
  - Q^T/K^T projections (bf16 matmul, fp32 PSUM) evacuated with fused
    bias-add + fp8e4 quantization (DVE tensor_scalar_add, fp8 out).
  - scores computed with fp8 DoubleRow matmuls: contraction d=64 fed as
    [64 part, 2(dup, stride 0), N] which doubles the result; the exp
    activation scale absorbs the factor 2.  Cost: 0.5 cycles/row.
  - exp on ScalarE from PSUM -> bf16 P in SBUF (the bottleneck engine:
    256 x [128,1024] activations).
  - PV in the FLIPPED orientation: out[q=128, 65] = P_tile.T @ [V | ones]
    using all 128 output partitions (2x the old [65, 512] form); the ones
    column yields the softmax denominator per query row, normalized during
    evacuation with a per-partition reciprocal + tensor_scalar multiply.
  - O (attention out, [q, d] layout) is PE-transposed back to O^T [d, q]
    for the output projection, which runs per query-half so only the last
    half is a serial tail.

Attention is emitted head-major per query-half (phase A = queries 0:1024
for all heads, phase B = queries 1024:2048), with the remaining
projections and phase-A's output projection interleaved into the PE
stream under the ACT-bound attention phases.
"""

import sys

if "/opt/trn_rl_repo" not in sys.path:
    sys.path.insert(0, "/opt/trn_rl_repo")

import numpy as np
import ml_dtypes

B, S, E, H = 4, 2048, 1024, 16
P = 128
HD = 64           # head dim
NH = 8            # heads per core
DT = 4            # d-tiles (head pairs) per core
ET = E // P       # 8 e-tiles (contraction for projections)
ST = S // P       # 16 key tiles
N_CORES = 8
QHALF = S // 2    # 1024
SCALE = 1.0 / float(np.sqrt(HD))

_BF16 = ml_dtypes.bfloat16

_cached = None


def _build():
    import concourse.bass as bass
    import concourse.tile as tile
    import concourse.mybir as mybir
    from concourse import bacc

    dt = mybir.dt
    nc = bacc.Bacc("TRN2", target_bir_lowering=False, debug=False)

    xt_d = nc.dram_tensor("xt", [E, S], dt.bfloat16, kind="ExternalInput").ap()
    wq_d = nc.dram_tensor("wq", [E, 512], dt.bfloat16, kind="ExternalInput").ap()
    wk_d = nc.dram_tensor("wk", [E, 512], dt.bfloat16, kind="ExternalInput").ap()
    wv_d = nc.dram_tensor("wv", [E, 512], dt.bfloat16, kind="ExternalInput").ap()
    wo_d = nc.dram_tensor("wo", [512, E], dt.bfloat16, kind="ExternalInput").ap()
    bq_d = nc.dram_tensor("bq", [P, DT], dt.float32, kind="ExternalInput").ap()
    bk_d = nc.dram_tensor("bk", [P, DT], dt.float32, kind="ExternalInput").ap()
    bv_d = nc.dram_tensor("bv", [1, 512], dt.bfloat16, kind="ExternalInput").ap()
    bo_d = nc.dram_tensor("bo", [P, ET], dt.float32, kind="ExternalInput").ap()
    iden_d = nc.dram_tensor("iden", [P, P], dt.bfloat16, kind="ExternalInput").ap()
    out_d = nc.dram_tensor("out", [E, S], dt.float32, kind="ExternalOutput").ap()

    DR = mybir.MatmulPerfMode.DoubleRow

    with tile.TileContext(nc) as tc:
        with (
            tc.tile_pool(name="const", bufs=1) as cpool,
            tc.tile_pool(name="acts", bufs=1) as apool,
            tc.tile_pool(name="pp", bufs=4) as ppool,        # P (exp out)
            tc.tile_pool(name="oqp", bufs=2) as oqpool,      # O [q, dd] staging
            tc.tile_pool(name="recp", bufs=2) as recpool,    # reciprocals
            tc.tile_pool(name="outs", bufs=4) as outpool,    # out staging
            tc.tile_pool(name="pssc", bufs=2, space="PSUM") as scpool,
            tc.tile_pool(name="pspv", bufs=1, space="PSUM") as pvpool,
            tc.tile_pool(name="pspj", bufs=1, space="PSUM") as pjpool,
            tc.tile_pool(name="pstp", bufs=1, space="PSUM") as tppool,
        ):
            # ---------------- constants / inputs -----------------------
            xt = cpool.tile([P, ET, S], dt.bfloat16)
            wq = cpool.tile([P, ET, 512], dt.bfloat16)
            wk = cpool.tile([P, ET, 512], dt.bfloat16)
            wv = cpool.tile([P, ET, 512], dt.bfloat16)
            wo = cpool.tile([P, DT, E], dt.bfloat16)
            bq = cpool.tile([P, DT], dt.float32)
            bk = cpool.tile([P, DT], dt.float32)
            bv = cpool.tile([1, 512], dt.bfloat16)
            bo = cpool.tile([P, ET], dt.float32)
            iden = cpool.tile([P, P], dt.bfloat16)
            ones1 = cpool.tile([1, P], dt.bfloat16)

            # activations
            qt8 = apool.tile([P, DT, S], dt.float8e4)   # Q^T (bias+fp8)
            kt8 = apool.tile([P, DT, S], dt.float8e4)   # K^T (bias+fp8)
            va = apool.tile([P, ST, NH, HD + 1], dt.bfloat16)  # V | ones
            scb = apool.tile([P, DT, S], dt.bfloat16)   # O^T (normalized)

            # DMAs: xt per e-tile for queue parallelism + early proj start
            nc.sync.dma_start(wk[:, :, :], wk_d.rearrange("(eo p) c -> p eo c", p=P))
            for e in range(ET):
                nc.sync.dma_start(xt[:, e, :], xt_d[e * P : (e + 1) * P, :])
            nc.sync.dma_start(wq[:, :, :], wq_d.rearrange("(eo p) c -> p eo c", p=P))
            nc.sync.dma_start(wv[:, :, :], wv_d.rearrange("(eo p) c -> p eo c", p=P))
            nc.sync.dma_start(bq[:], bq_d)
            nc.sync.dma_start(bk[:], bk_d)
            nc.sync.dma_start(bv[:], bv_d)
            nc.sync.dma_start(iden[:], iden_d)
            nc.sync.dma_start(wo[:], wo_d.rearrange("(eo p) c -> p eo c", p=P))
            nc.sync.dma_start(bo[:], bo_d)
            nc.gpsimd.memset(ones1[:], 1.0)
            nc.gpsimd.memset(va[:, :, :, HD : HD + 1], 1.0)

            # ---------------- projection emitters ----------------------
            def emit_k_proj(t, kc):
                """K^T for d-tile t, key chunk kc (512 keys) -> kt8."""
                ps = pjpool.tile([P, 512], dt.float32, tag="pj", name=f"k{t}{kc}")
                for e in range(ET):
                    nc.tensor.matmul(
                        ps[:],
                        wk[:, e, t * P : (t + 1) * P],
                        xt[:, e, kc * 512 : (kc + 1) * 512],
                        start=(e == 0),
                        stop=(e == ET - 1),
                    )
                nc.vector.tensor_scalar_add(
                    kt8[:, t, kc * 512 : (kc + 1) * 512], ps[:], bk[:, t : t + 1]
                )

            def emit_q_proj(t, qc):
                ps = pjpool.tile([P, 512], dt.float32, tag="pj", name=f"q{t}{qc}")
                for e in range(ET):
                    nc.tensor.matmul(
                        ps[:],
                        wq[:, e, t * P : (t + 1) * P],
                        xt[:, e, qc * 512 : (qc + 1) * 512],
                        start=(e == 0),
                        stop=(e == ET - 1),
                    )
                nc.vector.tensor_scalar_add(
                    qt8[:, t, qc * 512 : (qc + 1) * 512], ps[:], bq[:, t : t + 1]
                )

            def emit_v_proj(st):
                """V rows for key tile st (128 keys x 512 dims) -> va."""
                ps = pjpool.tile([P, 512], dt.float32, tag="pj", name=f"v{st}")
                for e in range(ET):
                    nc.tensor.matmul(
                        ps[:],
                        xt[:, e, st * P : (st + 1) * P],
                        wv[:, e, :],
                        start=(e == 0),
                        stop=False,
                    )
                nc.tensor.matmul(
                    ps[:], ones1[0:1, :], bv[0:1, :], start=False, stop=True
                )
                nc.vector.tensor_copy(
                    va[:, st, :, 0:HD], ps.rearrange("p (h d) -> p h d", d=HD)
                )

            # ---------------- attention emitter -------------------------
            def emit_attention(h, qh, extra_pe):
                """One head, one query half.  extra_pe: list of thunks emitting
                PE-side work to interleave into the j-loop (projections,
                outproj) -- consumed one per j step."""
                t, hp = h // 2, (h % 2) * HD
                # [q, qt-slot, 65 of 128] -- qt-stride of 512B keeps every
                # PV matmul inside one PSUM bank; col 64 is the denominator.
                # Zeroed up front so the accumulation needs no start=True
                # (start zeroes a whole 2KB zero-region, which would wipe
                # neighbouring qt groups).
                pv = pvpool.tile([P, 8, P], dt.float32, tag="pv", name=f"pv{h}{qh}")
                nc.vector.memset(pv[:], 0.0)
                ptiles = []
                for j in range(ST):
                    sc = scpool.tile([P, 1024], dt.float32, tag="sc", name=f"sc{h}{qh}{j}")
                    for qc in range(2):
                        q0 = qh * QHALF + qc * 512
                        nc.tensor.matmul(
                            sc[:, qc * 512 : (qc + 1) * 512],
                            kt8[hp : hp + HD, t, j * P : (j + 1) * P]
                            .unsqueeze(1)
                            .broadcast_to((HD, 2, P)),
                            qt8[hp : hp + HD, t, q0 : q0 + 512]
                            .unsqueeze(1)
                            .broadcast_to((HD, 2, 512)),
                            start=True,
                            stop=True,
                            perf_mode=DR,
                        )
                    p = ppool.tile([P, 1024], dt.bfloat16, tag="p", name=f"p{h}{qh}{j}")
                    # scale/2 compensates the stride-0 DoubleRow doubling
                    nc.scalar.activation(
                        p[:], sc[:], mybir.ActivationFunctionType.Exp, scale=SCALE / 2.0
                    )
                    ptiles.append(p)
                    for qt in range(8):
                        nc.tensor.matmul(
                            pv[:, qt, 0 : HD + 1],
                            p[:, qt * P : (qt + 1) * P],
                            va[:, j, h, :],
                            start=False,
                            stop=(j == ST - 1),
                            skip_group_check=True,
                        )
                    if extra_pe:
                        extra_pe.pop(0)()
                return pv

            def emit_evac(h, qh, pv, oq):
                """Normalize PV outputs into oq [q, dd] (dd-half per head)."""
                t, half = h // 2, h % 2
                rec = recpool.tile([P, 8], dt.float32, tag="rec", name=f"rc{h}{qh}")
                scr = recpool.tile([P, 8], dt.float32, tag="scr", name=f"sr{h}{qh}")
                nc.vector.reciprocal_approx_accurate(
                    rec[:], pv[:, :, HD : HD + 1].rearrange("p a b -> p (a b)"), scr[:]
                )
                for qt in range(8):
                    nc.vector.tensor_scalar(
                        oq[:, qt, half * HD : (half + 1) * HD],
                        pv[:, qt, 0:HD],
                        rec[:, qt : qt + 1],
                        None,
                        op0=mybir.AluOpType.mult,
                    )

            def emit_transpose(t, qh, oq):
                """oq [q, dd of pair t] -> scb[:, t, qh half] via PE transpose."""
                for qt in range(8):
                    tp = tppool.tile([P, P], dt.bfloat16, tag="tp", name=f"tp{t}{qh}{qt}")
                    nc.tensor.transpose(tp[:], oq[:, qt, :], iden[:])
                    q0 = qh * QHALF + qt * P
                    nc.vector.tensor_copy(scb[:, t, q0 : q0 + P], tp[:])

            def emit_outproj(eo, qh):
                """Partial out^T tile [128 Eo, 1024 q]: contract my 4 d-tiles.
                Shares the scores psum ring (same shape/tag)."""
                ps = scpool.tile([P, 1024], dt.float32, tag="sc", name=f"o{eo}{qh}")
                q0 = qh * QHALF
                for qc in range(2):
                    for t in range(DT):
                        nc.tensor.matmul(
                            ps[:, qc * 512 : (qc + 1) * 512],
                            wo[:, t, eo * P : (eo + 1) * P],
                            scb[:, t, q0 + qc * 512 : q0 + (qc + 1) * 512],
                            start=(t == 0),
                            stop=(t == DT - 1),
                        )
                ot = outpool.tile([P, 1024], dt.float32, tag="ot", name=f"oe{eo}{qh}")
                nc.vector.tensor_scalar_add(ot[:], ps[:], bo[:, eo : eo + 1])
                nc.sync.dma_start(out_d[eo * P : (eo + 1) * P, q0 : q0 + 1024], ot[:])

            # ---------------- emission schedule -------------------------
            # Pre-attention: K/Q projections for d-tile 0 (head 0+1), V st0/1.
            emit_k_proj(0, 0)
            emit_q_proj(0, 0)
            emit_q_proj(0, 1)
            emit_k_proj(0, 1)
            emit_v_proj(0)
            emit_v_proj(1)

            # Remaining projection work, interleaved into attention j-loops.
            # Each thunk is one PE chunk (~1.7us).
            proj_work = []
            for st in range(2, ST):
                proj_work.append(lambda st=st: emit_v_proj(st))
            for t in range(1, DT):
                proj_work.append(lambda t=t: emit_k_proj(t, 0))
                proj_work.append(lambda t=t: emit_k_proj(t, 1))
                proj_work.append(lambda t=t: emit_q_proj(t, 0))
                proj_work.append(lambda t=t: emit_q_proj(t, 1))
                proj_work.append(lambda t=t: emit_k_proj(t, 2))
                proj_work.append(lambda t=t: emit_k_proj(t, 3))
            # Q projections for the second query half (needed in phase B)
            for t in range(DT):
                proj_work.append(lambda t=t: emit_q_proj(t, 2))
                proj_work.append(lambda t=t: emit_q_proj(t, 3))

            # K chunks 2/3 of d-tile 0 must precede h0 attention j>=8
            emit_k_proj(0, 2)
            emit_k_proj(0, 3)

            # Phase A: heads 0..7 on query half 0
            oqs = {}
            for h in range(NH):
                t, half = h // 2, h % 2
                if half == 0:
                    oqs[t] = oqpool.tile([P, 8, P], dt.bfloat16, tag="oq", name=f"oq{t}A")
                pv = emit_attention(h, 0, proj_work)
                emit_evac(h, 0, pv, oqs[t])
                if half == 1:
                    emit_transpose(t, 0, oqs[t])

            # Phase B: heads 0..7 on query half 1, with phase-A outproj
            # interleaved once scb half 0 is complete (after h>=2 emission).
            outproj_work = [
                (lambda eo=eo: emit_outproj(eo, 0)) for eo in range(ET)
            ]
            for h in range(NH):
                t, half = h // 2, h % 2
                if half == 0:
                    oqs[t] = oqpool.tile([P, 8, P], dt.bfloat16, tag="oq", name=f"oq{t}B")
                extra = proj_work if proj_work else (outproj_work if h >= 2 else [])
                pv = emit_attention(h, 1, extra)
                emit_evac(h, 1, pv, oqs[t])
                if half == 1:
                    emit_transpose(t, 1, oqs[t])

            # Remaining phase-A outproj + all of phase-B outproj (tail)
            for w in outproj_work:
                w()
            for eo in range(ET):
                emit_outproj(eo, 1)

    nc.compile()
    return nc


def _prep_inputs(x, W_qkv, b_qkv, W_out, b_out):
    """Host-side sharding + layout prep. Returns per-core input maps."""
    w = W_qkv.reshape(E, H, 3, HD)
    b3 = b_qkv.reshape(H, 3, HD)
    iden = np.eye(P, dtype=np.float32).astype(_BF16)

    in_maps = []
    for core in range(N_CORES):
        b, hg = core // 2, core % 2
        hs = slice(hg * NH, (hg + 1) * NH)
        xt = np.ascontiguousarray(x[b].T).astype(_BF16)           # [E, S]
        wq = np.ascontiguousarray(w[:, hs, 0, :].reshape(E, 512)).astype(_BF16)
        wk = np.ascontiguousarray(w[:, hs, 1, :].reshape(E, 512)).astype(_BF16)
        wv = np.ascontiguousarray(w[:, hs, 2, :].reshape(E, 512)).astype(_BF16)
        wo = np.ascontiguousarray(W_out[hg * 512 : (hg + 1) * 512, :]).astype(_BF16)
        bq = np.ascontiguousarray(b3[hs, 0, :].reshape(DT, P).T).astype(np.float32)
        bk = np.ascontiguousarray(b3[hs, 1, :].reshape(DT, P).T).astype(np.float32)
        bv = np.ascontiguousarray(b3[hs, 2, :].reshape(1, 512)).astype(_BF16)
        bo = (np.ascontiguousarray(b_out.reshape(ET, P).T) * (1.0 if hg == 0 else 0.0)).astype(np.float32)
        in_maps.append(
            {
                "xt": xt,
                "wq": wq,
                "wk": wk,
                "wv": wv,
                "wo": wo,
                "bq": bq,
                "bk": bk,
                "bv": bv,
                "bo": bo,
                "iden": iden,
            }
        )
    return in_maps


def run_raw(x, W_qkv, b_qkv, W_out, b_out, trace=False, **kw):
    """Run on hardware; returns (full_output [B,S,E] f32, BassKernelResults)."""
    global _cached
    from concourse.bass_utils import run_bass_kernel_spmd

    if _cached is None:
        _cached = _build()
    nc = _cached
    in_maps = _prep_inputs(
        np.asarray(x), np.asarray(W_qkv), np.asarray(b_qkv),
        np.asarray(W_out), np.asarray(b_out),
    )
    res = run_bass_kernel_spmd(
        nc, in_maps, core_ids=list(range(N_CORES)), trace=trace, **kw
    )
    out = np.empty((B, S, E), dtype=np.float32)
    for b in range(B):
        acc = np.asarray(res.results[2 * b]["out"]) + np.asarray(
            res.results[2 * b + 1]["out"]
        )
        out[b] = acc.T
    return out, res


def kernel(x, W_qkv, b_qkv, W_out, b_out):
    out, _ = run_raw(x, W_qkv, b_qkv, W_out, b_out, trace=False)
    return out
